# revision 8
# baseline (speedup 1.0000x reference)
"""CBOW negative-sampling-style loss kernel for trn2, 8 NeuronCores.

Sharding:
  - batch (2048) data-parallel across 8 cores for the emb_v gathers / h
    computation / positive path (256 rows per core), then AllGather h.
  - vocab (50000) sharded across 8 cores (6250 rows each) for the negative
    h @ U^T matmul; per-row sigmoid sums are AllReduced before the log.

Per core:
  h_own[256,100]   = mean_ctx emb_v[x_shard]          (indirect DMA gathers)
  h[2048,100]      = AllGather(h_own)
  hT[100,2048]     = PE transpose, cast bf16
  scores           = hT[:,m]^T @ uT_shard  (bf16 matmul, PSUM [128,2048] spans)
  S_partial[b]     = sum_v sigmoid(-scores[b,v])      (ScalarE accum_out)
  sd[b]            = sigmoid(dot(emb_u[y_b], h_b))    (own batch shard only)
  AllReduce([2,2048]) -> S[b] (full vocab sum), sd[b] (full batch)
  loss             = mean_b( ln(S_b) - ln(sd_b) )
"""

import os
import numpy as np

import concourse.bass as bass
import concourse.bacc as bacc
import concourse.mybir as mybir
import concourse.tile as tile
from concourse.bass_utils import run_bass_kernel_spmd
from concourse.masks import make_identity

N_CORES = 8
V, E, B, CTX = 50000, 100, 2048, 10
VS = V // N_CORES     # 6250 vocab rows per core
BS = B // N_CORES     # 256 batch rows per core
P = 128
NB = B // P           # 16 batch tiles
NBS = BS // P         # 2 own batch tiles
GROUP = 2048          # PSUM span per ScalarE sigmoid call (4 banks)
NFULL = VS // GROUP   # 3 full groups
TAIL = VS - NFULL * GROUP  # 106
MMN = 512             # matmul moving free dim (one PSUM bank)

F32 = mybir.dt.float32
BF16 = mybir.dt.bfloat16
I32 = mybir.dt.int32

_last_results = None  # test harness reads exec_time_ns off this


def _build():
    nc = bacc.Bacc("TRN2", target_bir_lowering=False, debug=False,
                   num_devices=N_CORES)

    x_in = nc.dram_tensor("x", [BS, CTX], I32, kind="ExternalInput").ap()
    y_in = nc.dram_tensor("y", [BS, 1], I32, kind="ExternalInput").ap()
    embv = nc.dram_tensor("emb_v", [V, E], F32, kind="ExternalInput").ap()
    embu = nc.dram_tensor("emb_u", [V, E], F32, kind="ExternalInput").ap()
    ut_in = nc.dram_tensor("ut", [E, VS], F32, kind="ExternalInput").ap()
    onehot = nc.dram_tensor("onehot", [P, NB], F32, kind="ExternalInput").ap()
    loss_out = nc.dram_tensor("loss", [1, 1], F32, kind="ExternalOutput").ap()

    rg = [list(range(N_CORES))]

    with tile.TileContext(nc) as tc:
        with tc.tile_pool(name="dram", bufs=1, space="DRAM") as dram, \
             tc.tile_pool(name="sbuf", bufs=1) as sb, \
             tc.tile_pool(name="gp", bufs=3) as gp:

            h_own = dram.tile([BS, E], F32)
            h_full = dram.tile([B, E], F32)
            ar_in = dram.tile([2, B], F32)
            ar_out = dram.tile([2, B], F32)

            # --- uT load + bf16 cast (overlaps gather phase) ---
            ut_f = sb.tile([E, VS], F32)
            nc.sync.dma_start(out=ut_f[:], in_=ut_in[:])
            ut_b = sb.tile([E, VS], BF16)
            nc.vector.tensor_copy(ut_b[:], ut_f[:])

            ident = sb.tile([P, P], F32)
            make_identity(nc, ident[:])

            # --- own-shard h via gathers + positive path ---
            x_t = sb.tile([P, CTX * NBS], I32)
            y_t = sb.tile([P, NBS], I32)
            for t in range(NBS):
                nc.sync.dma_start(out=x_t[:, t * CTX:(t + 1) * CTX],
                                  in_=x_in[t * P:(t + 1) * P, :])
                nc.sync.dma_start(out=y_t[:, t:t + 1],
                                  in_=y_in[t * P:(t + 1) * P, :])

            sd = sb.tile([P, NBS], F32)   # sigmoid(pos dot), own tiles
            for t in range(NBS):
                hsum = gp.tile([P, E], F32, tag="hsum")
                for c in range(CTX):
                    g = gp.tile([P, E], F32, tag="gather")
                    nc.gpsimd.indirect_dma_start(
                        out=g[:], out_offset=None, in_=embv[:],
                        in_offset=bass.IndirectOffsetOnAxis(
                            ap=x_t[:, t * CTX + c: t * CTX + c + 1], axis=0))
                    if c == 0:
                        nc.vector.tensor_copy(hsum[:], g[:])
                    else:
                        nc.vector.tensor_add(hsum[:], hsum[:], g[:])
                nc.vector.tensor_scalar_mul(hsum[:], hsum[:], 1.0 / CTX)
                nc.sync.dma_start(out=h_own[t * P:(t + 1) * P, :], in_=hsum[:])

                # positive path: d = dot(emb_u[y], h) ; sd = sigmoid(d)
                uy = gp.tile([P, E], F32, tag="gather")
                nc.gpsimd.indirect_dma_start(
                    out=uy[:], out_offset=None, in_=embu[:],
                    in_offset=bass.IndirectOffsetOnAxis(
                        ap=y_t[:, t:t + 1], axis=0))
                prod = sb.tile([P, E], F32, tag="prod")
                nc.vector.tensor_mul(prod[:], uy[:], hsum[:])
                d = sb.tile([P, 1], F32, tag="dvec")
                nc.vector.tensor_reduce(d[:], prod[:],
                                        axis=mybir.AxisListType.X,
                                        op=mybir.AluOpType.add)
                nc.scalar.activation(sd[:, t:t + 1], d[:],
                                     mybir.ActivationFunctionType.Sigmoid)

            # --- AllGather h ---
            nc.gpsimd.collective_compute(
                "AllGather", mybir.AluOpType.bypass, replica_groups=rg,
                ins=[h_own.opt()], outs=[h_full.opt()])

            # --- hT build: 16 transposes ---
            hT = sb.tile([E, B], BF16)
            with tc.tile_pool(name="tp_psum", bufs=2, space="PSUM") as tpp:
                for m in range(NB):
                    hm = gp.tile([P, E], F32, tag="hm")
                    nc.sync.dma_start(out=hm[:],
                                      in_=h_full[m * P:(m + 1) * P, :])
                    tp = tpp.tile([E, P], F32)
                    nc.tensor.transpose(tp[:], hm[:], ident[:])
                    nc.vector.tensor_copy(hT[:, m * P:(m + 1) * P], tp[:])

            # --- main loop: scores -> sigmoid -> row sums ---
            S_part = sb.tile([P, NB], F32)
            sig_scr = sb.tile([P, GROUP], BF16)
            groups = [(i * GROUP, GROUP) for i in range(NFULL)]
            if TAIL:
                groups.append((NFULL * GROUP, TAIL))
            with tc.tile_pool(name="mm_psum", bufs=2, space="PSUM") as mmp:
                for m in range(NB):
                    lhsT = hT[:, m * P:(m + 1) * P]
                    acc4 = gp.tile([P, len(groups)], F32, tag="acc4")
                    for gi, (v0, vn) in enumerate(groups):
                        pg = mmp.tile([P, GROUP], F32)
                        for n0 in range(0, vn, MMN):
                            nn = min(MMN, vn - n0)
                            nc.tensor.matmul(
                                pg[:, n0:n0 + nn], lhsT,
                                ut_b[:, v0 + n0: v0 + n0 + nn],
                                start=True, stop=True)
                        nc.scalar.activation(
                            sig_scr[:, :vn], pg[:, :vn],
                            mybir.ActivationFunctionType.Sigmoid,
                            scale=-1.0, accum_out=acc4[:, gi:gi + 1])
                    nc.vector.tensor_reduce(S_part[:, m:m + 1], acc4[:],
                                            axis=mybir.AxisListType.X,
                                            op=mybir.AluOpType.add)

            # --- pack AllReduce input ---
            # row0: S_part flattened p-major (b' = p*16 + m), full batch
            # row1: sd one-hot masked into own columns, zeros elsewhere
            # srow[:, m] = sd[:, t] if m == 2*core+t else 0. The program is
            # SPMD (one compile for all cores), so the core-dependent column
            # choice comes from the host-passed onehot mask: replicate sd
            # across all 16 columns, then multiply by the mask.
            srow = sb.tile([P, NB], F32)
            oh = sb.tile([P, NB], F32)
            nc.sync.dma_start(out=oh[:], in_=onehot[:])
            sd_exp = sb.tile([P, NB], F32)
            for a in range(NB // NBS):
                nc.vector.tensor_copy(sd_exp[:, a * NBS:(a + 1) * NBS], sd[:])
            nc.vector.tensor_mul(srow[:], sd_exp[:], oh[:])

            nc.sync.dma_start(
                out=ar_in[0, :].rearrange("(p m) -> p m", p=P), in_=S_part[:])
            nc.sync.dma_start(
                out=ar_in[1, :].rearrange("(p m) -> p m", p=P), in_=srow[:])

            nc.gpsimd.collective_compute(
                "AllReduce", mybir.AluOpType.add, replica_groups=rg,
                ins=[ar_in.opt()], outs=[ar_out.opt()])

            # --- final: loss = mean_b( ln(S_b) - ln(sd_b) ) ---
            Sf = sb.tile([P, NB], F32)
            Gf = sb.tile([P, NB], F32)
            nc.sync.dma_start(
                out=Sf[:], in_=ar_out[0, :].rearrange("(p m) -> p m", p=P))
            nc.sync.dma_start(
                out=Gf[:], in_=ar_out[1, :].rearrange("(p m) -> p m", p=P))
            Gr = sb.tile([P, NB], F32)
            nc.vector.reciprocal(Gr[:], Gf[:])
            R = sb.tile([P, NB], F32)
            nc.vector.tensor_mul(R[:], Sf[:], Gr[:])
            L = sb.tile([P, NB], F32)
            nc.scalar.activation(L[:], R[:], mybir.ActivationFunctionType.Ln)
            Lr = sb.tile([P, 1], F32)
            nc.vector.tensor_reduce(Lr[:], L[:], axis=mybir.AxisListType.X,
                                    op=mybir.AluOpType.add)
            ones = sb.tile([P, 1], F32)
            nc.vector.memset(ones[:], 1.0)
            with tc.tile_pool(name="fin_psum", bufs=1, space="PSUM") as fpp:
                lp = fpp.tile([1, 1], F32)
                nc.tensor.matmul(lp[:], ones[:], Lr[:], start=True, stop=True)
                ls = sb.tile([1, 1], F32)
                nc.scalar.mul(ls[:], lp[:], 1.0 / B)
                nc.sync.dma_start(out=loss_out[:], in_=ls[:])

    nc.compile()
    return nc


_nc_cache = None


def kernel(x_positive, y, emb_v, emb_u):
    global _nc_cache, _last_results
    x32 = np.ascontiguousarray(np.asarray(x_positive, dtype=np.int32))
    y32 = np.ascontiguousarray(np.asarray(y, dtype=np.int32)).reshape(B, 1)
    ev = np.ascontiguousarray(np.asarray(emb_v, dtype=np.float32))
    eu = np.ascontiguousarray(np.asarray(emb_u, dtype=np.float32))

    if _nc_cache is None:
        _nc_cache = _build()
    nc = _nc_cache

    in_maps = []
    for c in range(N_CORES):
        oh = np.zeros((P, NB), dtype=np.float32)
        oh[:, 2 * c] = 1.0
        oh[:, 2 * c + 1] = 1.0
        in_maps.append({
            "x": x32[c * BS:(c + 1) * BS, :],
            "y": y32[c * BS:(c + 1) * BS, :],
            "emb_v": ev,
            "emb_u": eu,
            "ut": np.ascontiguousarray(eu[c * VS:(c + 1) * VS, :].T),
            "core_id": np.array([[float(c)]], dtype=np.float32),
            "onehot": oh,
        })

    trace = bool(os.environ.get("BASS_TRACE"))
    res = run_bass_kernel_spmd(nc, in_maps, list(range(N_CORES)), trace=trace)
    _last_results = res
    loss = res.results[0]["loss"][0, 0]
    return np.asarray(loss, dtype=np.float32).reshape(())


# revision 11
# speedup vs baseline: 1.1996x; 1.1996x over previous
"""CBOW negative-sampling-style loss kernel for trn2, 8 NeuronCores.

Sharding:
  - batch (2048) data-parallel across 8 cores for the emb_v gathers / h
    computation / positive path. Each core owns batch tiles m=c and m=8+c
    (rows [c*128:(c+1)*128] and [1024+c*128:1024+(c+1)*128]) so h can be
    exchanged in TWO AllGathers: the main matmul loop on tiles 0-7 starts
    while the second gather wave + AllGather are still in flight.
  - vocab (50000) sharded across 8 cores (6250 rows each) for the negative
    h @ U^T matmul; per-row sigmoid sums are AllReduced before the log.

Per core:
  h_own[2][128,128]  = mean_ctx emb_v[x_shard] (bf16, E padded 100->128)
  h[2048,128]        = 2x AllGather(h_own)
  hT[128,2048]       = one xbar DMA-transpose per AllGather half
  scores             = hT[:,m]^T @ uT_shard  (bf16 matmul, PSUM [128,2048])
  S_partial[b]       = sum_v sigmoid(-scores[b,v])   (ScalarE accum_out)
  sd[b]              = sigmoid(dot(emb_u[y_b], h_b)) (own batch rows)
  AllReduce([2,2048]) -> S[b] (full vocab sum), sd[b] (full batch)
  loss               = mean_b( ln(S_b) - ln(sd_b) )
"""

import os
import numpy as np

import concourse.bass as bass
import concourse.bacc as bacc
import concourse.mybir as mybir
import concourse.tile as tile
from concourse.bass_utils import run_bass_kernel_spmd

N_CORES = 8
V, E, B, CTX = 50000, 100, 2048, 10
EP = 128              # E padded to full partition dim
VS = V // N_CORES     # 6250 vocab rows per core
BS = B // N_CORES     # 256 batch rows per core (2 tiles of 128)
P = 128
NB = B // P           # 16 batch tiles
NBS = BS // P         # 2 own batch tiles (phases A and B)
HALF = B // 2         # 1024 rows per AllGather half
GROUP = 2048          # PSUM span per ScalarE sigmoid call (4 banks)
NFULL = VS // GROUP   # 3 full groups
TAIL = VS - NFULL * GROUP  # 106
MMN = 512             # matmul moving free dim (one PSUM bank)

F32 = mybir.dt.float32
BF16 = mybir.dt.bfloat16
I32 = mybir.dt.int32

_last_results = None  # test harness reads exec_time_ns off this


def _build():
    nc = bacc.Bacc("TRN2", target_bir_lowering=False, debug=False,
                   num_devices=N_CORES)

    x_in = nc.dram_tensor("x", [BS, CTX], I32, kind="ExternalInput").ap()
    y_in = nc.dram_tensor("y", [BS, 1], I32, kind="ExternalInput").ap()
    embv = nc.dram_tensor("emb_v", [V, E], F32, kind="ExternalInput").ap()
    embu = nc.dram_tensor("emb_u", [V, E], F32, kind="ExternalInput").ap()
    ut_in = nc.dram_tensor("ut", [E, VS], F32, kind="ExternalInput").ap()
    onehot = nc.dram_tensor("onehot", [P, NB], F32, kind="ExternalInput").ap()
    loss_out = nc.dram_tensor("loss", [1, 1], F32, kind="ExternalOutput").ap()

    rg = [list(range(N_CORES))]

    with tile.TileContext(nc) as tc:
        with tc.tile_pool(name="dram", bufs=1, space="DRAM") as dram, \
             tc.tile_pool(name="sbuf", bufs=1) as sb, \
             tc.tile_pool(name="gp", bufs=3) as gp:

            ar_in = dram.tile([2, B], F32)
            ar_out = dram.tile([2, B], F32)

            # x/y first on the sync HWDGE ring so the gathers can start
            # immediately; the big ut load goes on the scalar ring.
            x_t = sb.tile([P, CTX * NBS], I32)
            y_t = sb.tile([P, NBS], I32)
            for t in range(NBS):
                nc.sync.dma_start(out=x_t[:, t * CTX:(t + 1) * CTX],
                                  in_=x_in[t * P:(t + 1) * P, :])
                nc.sync.dma_start(out=y_t[:, t:t + 1],
                                  in_=y_in[t * P:(t + 1) * P, :])

            ut_f = sb.tile([E, VS], F32)
            nc.scalar.dma_start(out=ut_f[:], in_=ut_in[:])
            ut_b = sb.tile([EP, VS], BF16)
            # engine APs need 32-aligned base partition: zero [96:128] first,
            # the cast then overwrites [96:100]
            nc.vector.memset(ut_b[96:EP, :], 0.0)
            nc.vector.tensor_copy(ut_b[:E, :], ut_f[:])

            hT = sb.tile([EP, B], BF16)
            sd = sb.tile([P, NBS], F32)   # sigmoid(pos dot), own tiles

            # --- per-phase: gathers -> h -> pos path -> AllGather half ---
            for t in range(NBS):
                hsum = gp.tile([P, E], F32, tag="hsum")
                for c in range(CTX):
                    g = gp.tile([P, E], F32, tag="gather")
                    nc.gpsimd.indirect_dma_start(
                        out=g[:], out_offset=None, in_=embv[:],
                        in_offset=bass.IndirectOffsetOnAxis(
                            ap=x_t[:, t * CTX + c: t * CTX + c + 1], axis=0))
                    if c == 0:
                        nc.vector.tensor_copy(hsum[:], g[:])
                    else:
                        nc.vector.tensor_add(hsum[:], hsum[:], g[:])
                nc.vector.tensor_scalar_mul(hsum[:], hsum[:], 1.0 / CTX)

                # positive path: d = dot(emb_u[y], h) ; sd = sigmoid(d)
                uy = gp.tile([P, E], F32, tag="gather")
                nc.gpsimd.indirect_dma_start(
                    out=uy[:], out_offset=None, in_=embu[:],
                    in_offset=bass.IndirectOffsetOnAxis(
                        ap=y_t[:, t:t + 1], axis=0))
                prod = gp.tile([P, E], F32, tag="prod")
                nc.vector.tensor_mul(prod[:], uy[:], hsum[:])
                d = gp.tile([P, 1], F32, tag="dvec")
                nc.vector.tensor_reduce(d[:], prod[:],
                                        axis=mybir.AxisListType.X,
                                        op=mybir.AluOpType.add)
                nc.scalar.activation(sd[:, t:t + 1], d[:],
                                     mybir.ActivationFunctionType.Sigmoid)

                # bf16 cast, pad E->128, ship out, AllGather this half
                hbf = gp.tile([P, EP], BF16, tag="hbf")
                nc.vector.memset(hbf[:, E:EP], 0.0)
                nc.vector.tensor_copy(hbf[:, :E], hsum[:])
                h_own = dram.tile([P, EP], BF16, tag=f"h_own{t}")
                nc.sync.dma_start(out=h_own[:], in_=hbf[:])
                h_half = dram.tile([HALF, EP], BF16, tag=f"h_half{t}")
                nc.gpsimd.collective_compute(
                    "AllGather", mybir.AluOpType.bypass, replica_groups=rg,
                    ins=[h_own.opt()], outs=[h_half.opt()])
                nc.sync.dma_start_transpose(
                    out=hT[:, t * HALF:(t + 1) * HALF], in_=h_half[:])

            # --- main loop: scores -> sigmoid -> row sums ---
            S_part = sb.tile([P, NB], F32)
            sig_scr = sb.tile([P, GROUP], BF16)
            groups = [(i * GROUP, GROUP) for i in range(NFULL)]
            if TAIL:
                groups.append((NFULL * GROUP, TAIL))
            with tc.tile_pool(name="mm_psum", bufs=2, space="PSUM") as mmp:
                for m in range(NB):
                    lhsT = hT[:, m * P:(m + 1) * P]
                    acc4 = gp.tile([P, len(groups)], F32, tag="acc4")
                    for gi, (v0, vn) in enumerate(groups):
                        pg = mmp.tile([P, GROUP], F32)
                        for n0 in range(0, vn, MMN):
                            nn = min(MMN, vn - n0)
                            nc.tensor.matmul(
                                pg[:, n0:n0 + nn], lhsT,
                                ut_b[:, v0 + n0: v0 + n0 + nn],
                                start=True, stop=True)
                        nc.scalar.activation(
                            sig_scr[:, :vn], pg[:, :vn],
                            mybir.ActivationFunctionType.Sigmoid,
                            scale=-1.0, accum_out=acc4[:, gi:gi + 1])
                    nc.vector.tensor_reduce(S_part[:, m:m + 1], acc4[:],
                                            axis=mybir.AxisListType.X,
                                            op=mybir.AluOpType.add)

            # --- pack AllReduce input ---
            # row0: S_part flattened p-major (b' = p*16 + m), full batch
            # row1: sd replicated over columns, masked by host onehot
            srow = sb.tile([P, NB], F32)
            oh = sb.tile([P, NB], F32)
            nc.sync.dma_start(out=oh[:], in_=onehot[:])
            sd_exp = sb.tile([P, NB], F32)
            for a in range(NB // 2):
                nc.vector.tensor_copy(sd_exp[:, a:a + 1], sd[:, 0:1])
                nc.vector.tensor_copy(sd_exp[:, NB // 2 + a:NB // 2 + a + 1],
                                      sd[:, 1:2])
            nc.vector.tensor_mul(srow[:], sd_exp[:], oh[:])

            nc.sync.dma_start(
                out=ar_in[0, :].rearrange("(p m) -> p m", p=P), in_=S_part[:])
            nc.sync.dma_start(
                out=ar_in[1, :].rearrange("(p m) -> p m", p=P), in_=srow[:])

            nc.gpsimd.collective_compute(
                "AllReduce", mybir.AluOpType.add, replica_groups=rg,
                ins=[ar_in.opt()], outs=[ar_out.opt()])

            # --- final: loss = mean_b( ln(S_b) - ln(sd_b) ) ---
            Sf = sb.tile([P, NB], F32)
            Gf = sb.tile([P, NB], F32)
            nc.sync.dma_start(
                out=Sf[:], in_=ar_out[0, :].rearrange("(p m) -> p m", p=P))
            nc.sync.dma_start(
                out=Gf[:], in_=ar_out[1, :].rearrange("(p m) -> p m", p=P))
            Gr = sb.tile([P, NB], F32)
            nc.vector.reciprocal(Gr[:], Gf[:])
            R = sb.tile([P, NB], F32)
            nc.vector.tensor_mul(R[:], Sf[:], Gr[:])
            L = sb.tile([P, NB], F32)
            nc.scalar.activation(L[:], R[:], mybir.ActivationFunctionType.Ln)
            Lr = sb.tile([P, 1], F32)
            nc.vector.tensor_reduce(Lr[:], L[:], axis=mybir.AxisListType.X,
                                    op=mybir.AluOpType.add)
            ones = sb.tile([P, 1], F32)
            nc.vector.memset(ones[:], 1.0)
            with tc.tile_pool(name="fin_psum", bufs=1, space="PSUM") as fpp:
                lp = fpp.tile([1, 1], F32)
                nc.tensor.matmul(lp[:], ones[:], Lr[:], start=True, stop=True)
                ls = sb.tile([1, 1], F32)
                nc.scalar.mul(ls[:], lp[:], 1.0 / B)
                nc.sync.dma_start(out=loss_out[:], in_=ls[:])

    nc.compile()
    return nc


_nc_cache = None


def kernel(x_positive, y, emb_v, emb_u):
    global _nc_cache, _last_results
    x32 = np.ascontiguousarray(np.asarray(x_positive, dtype=np.int32))
    y32 = np.ascontiguousarray(np.asarray(y, dtype=np.int32)).reshape(B, 1)
    ev = np.ascontiguousarray(np.asarray(emb_v, dtype=np.float32))
    eu = np.ascontiguousarray(np.asarray(emb_u, dtype=np.float32))

    if _nc_cache is None:
        _nc_cache = _build()
    nc = _nc_cache

    in_maps = []
    for c in range(N_CORES):
        # core c owns batch tiles m=c (phase A) and m=8+c (phase B)
        rows = np.r_[c * P:(c + 1) * P, HALF + c * P:HALF + (c + 1) * P]
        oh = np.zeros((P, NB), dtype=np.float32)
        oh[:, c] = 1.0        # phase A tile -> sd[:, 0]
        oh[:, NB // 2 + c] = 1.0  # phase B tile -> sd[:, 1]
        in_maps.append({
            "x": x32[rows, :],
            "y": y32[rows, :],
            "emb_v": ev,
            "emb_u": eu,
            "ut": np.ascontiguousarray(eu[c * VS:(c + 1) * VS, :].T),
            "onehot": oh,
        })

    trace = bool(os.environ.get("BASS_TRACE"))
    res = run_bass_kernel_spmd(nc, in_maps, list(range(N_CORES)), trace=trace)
    _last_results = res
    loss = res.results[0]["loss"][0, 0]
    return np.asarray(loss, dtype=np.float32).reshape(())
